# revision 32
# baseline (speedup 1.0000x reference)
"""Self-contained Trainium2 Bass kernel for the 3-layer LSTM problem
(nn_CustomModel_16681652978184): T=4096, B=6, F=128, H1=512, H3=128.

Strategy (chosen over the sharding hint's per-step tensor-parallel option):
the recurrence is strictly serial (8192 dependent steps: L2's initial state
is L1's *final* state, so L1/L2 cannot pipeline), and cross-core exchange
floors on trn2 (~2us DMA fixed cost, ~5-10us collective floor) dwarf the
~4us per-step compute -- an "all-reduce h each step" design would spend
40ms+ in sync alone.  So the serial recurrence runs on ONE NeuronCore,
structured to make each step as fast as the PE weight-load bandwidth allows:

  - "Transposed land": activations live as [H-on-partitions, batch].
    Recurrent matmul z^T = Wh^T @ h^T with bf16 weight chunks stationary
    (Fast-Weight-Load) and tiny h^T [128, 6] moving operands.
  - Gate-column permutation: PSUM gate tiles hold (i | f | o | g) x batch
    per H-block, and gate math is emitted as single strided-AP instructions
    spanning all blocks of a half (sigmoid: one [128, nb/2, 18] ACT op) --
    the ACT fixed cost (~300ns/instr) makes many tiny ops ruinous.
  - Half-split software pipelining: the H-blocks are split in two halves;
    while the PE streams half B's matmuls, half A's gate chain runs on
    ACT/VEC, hiding the serial gate latency under the weight stream.
  - Input projections (x @ Wi + b) computed just-in-time inside the loop
    body (off the critical path) into SBUF ring buffers; only seq1 round
    trips through DRAM (25MB > SBUF).  L3 trails L2 by one body: its input
    projections (Wi3^T h2 + b3) are batched per half-body (jit_z3x, N=192
    moving operand -> 0.5 weight-chunk loads/step instead of 16), and only
    the 4 Wh3 chunks remain per-step; l3 epilogue drains the final body.
  - Wh1/Wh2 are fp8 e4m3 (trn2 float8e4: max finite 240) scaled by 2^12,
    halving the dominant LDWEIGHTS stream; Wi/b of L1/L2 carry the same
    scale (bf16/f32) and the descale rides the gate ACT scale= operand for
    free.  L3 + Wl stay bf16 (L3 quantization fails the 2e-2 gate).
    Measured rel err 1.1e-2 (gate 2e-2).
  - Dynamic For_i outer loops with unrolled bodies; parity-free ring
    buffers keep all inner addressing static.  TREP env knob wraps both
    phases in an outer repeat loop (state is re-zeroed each round) for
    overhead-cancelling slope timing; semantics are unchanged (TREP=1).
"""

import os
import numpy as np
import ml_dtypes

import concourse.bass as bass
import concourse.mybir as mybir
from concourse import bacc, tile
from concourse.bass_utils import run_bass_kernel_spmd

F32 = mybir.dt.float32
BF16 = mybir.dt.bfloat16
AF = mybir.ActivationFunctionType

P = 128
BSZ = 6

T_FULL = 4096
BODY_DEFAULT = 64

# Wh1/Wh2 are stored fp8 e4m3 (trn2 float8e4 = IEEE-ish: max finite 240,
# exp-15 encodes inf/nan) scaled by 2^12: |Wh|<=1/sqrt(512)*4096=181<240.
# Wi/b of those layers carry the same scale so zsum is uniformly scaled; the
# descale folds into the gate activations' scale= operand (zero extra instrs).
GSCALE = 2.0 ** 12
DESCALE = 2.0 ** -12

# slot -> reference gate column-block base multiplier (ref order i,f,g,o)
_SLOT_BASE = {0: 0, 1: 1, 2: 3, 3: 2}  # our slots: i, f, o, g


def gcol(H, kb, s):
    return _SLOT_BASE[s] * H + kb * P


def prep_layer(Wi, Wh, b, H, wh_fp8=False):
    bf = ml_dtypes.bfloat16
    nb = H // P
    KCi = Wi.shape[0] // P
    KCh = Wh.shape[0] // P
    scale = GSCALE
    wh_dt = ml_dtypes.float8_e4m3 if wh_fp8 else bf
    Wi = np.asarray(Wi) * scale
    Wh = np.asarray(Wh) * scale
    b = np.asarray(b) * scale
    WiP = np.zeros((P, nb * 4 * KCi * P), dtype=bf)
    WhP = np.zeros((P, nb * 4 * KCh * P), dtype=wh_dt)
    bP = np.zeros((P, nb * 4), dtype=np.float32)
    for kb in range(nb):
        for s in range(4):
            col = gcol(H, kb, s)
            bP[:, kb * 4 + s] = b[col:col + P]
            for kc in range(KCi):
                idx = ((kb * 4 + s) * KCi + kc) * P
                WiP[:, idx:idx + P] = Wi[kc * P:(kc + 1) * P, col:col + P].astype(bf)
            for kc in range(KCh):
                idx = ((kb * 4 + s) * KCh + kc) * P
                WhP[:, idx:idx + P] = Wh[kc * P:(kc + 1) * P, col:col + P].astype(wh_dt)
    return WiP, WhP, bP


def prep_inputs(inp, T, BODY):
    bf = ml_dtypes.bfloat16
    x = np.asarray(inp["x"])[:T]
    Tpad = T + 2 * BODY
    xT = np.zeros((P, Tpad * BSZ), dtype=bf)
    xT[:, : T * BSZ] = x.reshape(T * BSZ, P).T.astype(bf)

    Wi1P, Wh1P, b1P = prep_layer(inp["Wi1"], inp["Wh1"], inp["b1"], 512, wh_fp8=True)
    Wi2P, Wh2P, b2P = prep_layer(inp["Wi2"], inp["Wh2"], inp["b2"], 512, wh_fp8=True)
    Wi3P, Wh3P, b3P = prep_layer(inp["Wi3"], inp["Wh3"], inp["b3"], 128)
    # broadcast b3 over batch for the fused-L3 gate add: [128, 4slots*6]
    b3bc = np.repeat(b3P[:, 0:4], BSZ, axis=1).astype(np.float32)
    WlP = np.asarray(inp["Wl"]).astype(bf)
    return {
        "xT": xT,
        "Wi1P": Wi1P, "Wh1P": Wh1P, "b1P": b1P,
        "Wi2P": Wi2P, "Wh2P": Wh2P, "b2P": b2P,
        "Wi3P": Wi3P, "Wh3P": Wh3P, "b3bc": b3bc,
        "WlP": WlP,
    }, float(np.asarray(inp["bl"])[0])


def build_lstm(tc, outs, ins, T, BODY, bl_value):
    nc = tc.nc
    assert T % BODY == 0 and BODY % 2 == 0
    HB = BODY // 2
    NBODY = T // BODY
    Tpad = T + 2 * BODY

    from contextlib import ExitStack
    ctx = ExitStack()
    const = ctx.enter_context(tc.tile_pool(name="const", bufs=1))
    state = ctx.enter_context(tc.tile_pool(name="state", bufs=1))
    ppool = ctx.enter_context(tc.tile_pool(name="ppool", bufs=1, space=bass.MemorySpace.PSUM))
    jitp = ctx.enter_context(tc.tile_pool(name="jitp", bufs=2, space=bass.MemorySpace.PSUM))
    dram = ctx.enter_context(tc.tile_pool(name="dram", bufs=1, space=bass.MemorySpace.DRAM))
    work = ctx.enter_context(tc.tile_pool(name="work", bufs=4))

    def load_const(key, shape, dtype):
        t = const.tile(shape, dtype, tag=key, name=key)
        nc.sync.dma_start(t[:], ins[key])
        return t

    xT = load_const("xT", [P, Tpad * BSZ], BF16)
    F8 = mybir.dt.float8e4
    W = {}
    for L, KCi, KCh, nb in ((1, 1, 4, 4), (2, 4, 4, 4), (3, 4, 1, 1)):
        W[L] = dict(
            wi=load_const(f"Wi{L}P", [P, nb * 4 * KCi * P], BF16),
            wh=load_const(f"Wh{L}P", [P, nb * 4 * KCh * P], F8 if L != 3 else BF16),
            KCi=KCi, KCh=KCh, nb=nb,
        )
    W[1]["b"] = load_const("b1P", [P, 16], F32)
    W[2]["b"] = load_const("b2P", [P, 16], F32)
    b3bc = load_const("b3bc", [P, 24], F32)
    wl = load_const("WlP", [P, 1], BF16)

    # block 4 of h/c = the fused L3 recurrence (one body behind L2): its gate
    # math rides the half-B strided instructions for free.
    hA = state.tile([P, 5, HB, BSZ], BF16, tag="hA")
    hB = state.tile([P, 5, HB, BSZ], BF16, tag="hB")
    cA = state.tile([P, 5, BSZ], F32, tag="cA")
    cB = state.tile([P, 5, BSZ], F32, tag="cB")
    zxR = [state.tile([P, 5, HB, 24], F32, tag=f"zxR{i}", name=f"zxR{i}") for i in range(2)]
    S = [state.tile([P, 4, HB * BSZ], BF16, tag=f"S{i}", name=f"S{i}") for i in range(2)]
    zpad = state.tile([P, 4 * 2 * BODY * BSZ], BF16, tag="zpad")

    # PSUM: 2 halves x (lo=kc01 | hi=kc23) + L3 + jit pool (2) = 7 banks.
    # lo/hi split keeps accumulation groups consecutive (interleaved groups
    # corrupt PSUM) while letting the first 16 MMs of a step depend only on
    # the previous step's first-half h.  (Merging lo+hi into single 4-chunk
    # groups was tried: +10ms -- the early-start overlap is load-bearing.)
    zplo = [ppool.tile([P, 48 + 24 * h], F32, tag=f"zplo{h}", name=f"zplo{h}") for h in (0, 1)]
    zphi = [ppool.tile([P, 48 + 24 * h], F32, tag=f"zphi{h}", name=f"zphi{h}") for h in (0, 1)]

    seq1T = dram.tile([P, 4, Tpad * BSZ], BF16, tag="seq1T")

    # =====================================================================
    def jit_zx(L, dst, base, Ssrc=None):
        """zx (= Wi^T @ input + b) for HB steps starting at absolute step
        `base` (int or ScalarValue) into dst [P, nb, HB, 24] (bf16)."""
        w = W[L]
        for kb in range(w["nb"]):
            for s in range(4):
                pt = jitp.tile([P, HB * BSZ], F32, tag="jit", name="jit")
                for kc in range(w["KCi"]):
                    if L == 1:
                        rhs = xT[:, bass.ds(base * BSZ, HB * BSZ)]
                    else:
                        rhs = Ssrc[:, kc, :]
                    idx = ((kb * 4 + s) * w["KCi"] + kc) * P
                    nc.tensor.matmul(
                        pt[:], w["wi"][:, idx:idx + P], rhs,
                        start=(kc == 0), stop=(kc == w["KCi"] - 1))
                nc.vector.tensor_scalar_add(
                    dst[:, kb, :, 6 * s:6 * s + 6],
                    pt[:].rearrange("p (t b) -> p t b", b=BSZ),
                    w["b"][:, kb * 4 + s:kb * 4 + s + 1])

    def step_mms(L, half, st, h_prev):
        """PE stream for one half of step st, split into lo (kc 0-1) and hi
        (kc 2-3) accumulators so the lo block only needs h-blocks 0-1 of the
        previous step (whose gate chain finished earliest)."""
        w = W[L]
        KCh = w["KCh"]
        assert KCh == 4
        groups = [(zplo[half], (0, 1)), (zphi[half], (2, 3))]
        for zp, kcs in groups:
            for kb in (half * 2, half * 2 + 1):
                for s in range(4):
                    o = 24 * (kb - half * 2) + 6 * s
                    for j, kc in enumerate(kcs):
                        idx = ((kb * 4 + s) * KCh + kc) * P
                        nc.tensor.matmul(
                            zp[:, o:o + 6],
                            w["wh"][:, idx:idx + P],
                            h_prev[:, kc, :],
                            start=(j == 0), stop=(j == len(kcs) - 1))

    def gates_half(L, half, st, h_cur, c_prev, c_cur, zx_ap, z3x_ap=None):
        """Gate math for blocks [2*half, 2*half+2) of step st, merged into
        strided single instructions.  When z3x_ap is given (fused L3 on
        half 1), a third block rides every instruction: slot 2 of the zsum
        tile is filled from z3p + z3x and blocks [2:5) of h/c are updated."""
        k0 = half * 2
        nb3 = 3 if z3x_ap is not None else 2
        lo3 = zplo[half][:].rearrange("p (k g) -> p k g", g=24)
        hi3 = zphi[half][:].rearrange("p (k g) -> p k g", g=24)
        zsum = work.tile([P, nb3, 24], F32, tag=f"zsum{nb3}", name="zsum")
        zs0 = work.tile([P, nb3, 24], F32, tag=f"zs0{nb3}", name="zs0")
        nc.vector.tensor_add(zs0[:], lo3[:, 0:nb3, :], zx_ap[:, k0:k0 + nb3, :])
        nc.vector.tensor_add(zsum[:], zs0[:], hi3[:, 0:nb3, :])
        sig = work.tile([P, nb3, 18], F32, tag=f"sig{nb3}", name="sig")
        nc.scalar.activation(sig[:], zsum[:, :, 0:18], AF.Sigmoid, scale=DESCALE)
        tg = work.tile([P, nb3, BSZ], F32, tag=f"tg{nb3}", name="tg")
        nc.scalar.activation(tg[:], zsum[:, :, 18:24], AF.Tanh, scale=DESCALE)
        m1 = work.tile([P, nb3, BSZ], F32, tag=f"m1{nb3}", name="m1")
        nc.vector.tensor_mul(m1[:], sig[:, :, 6:12], c_prev[:, k0:k0 + nb3, :])
        m2 = work.tile([P, nb3, BSZ], F32, tag=f"m2{nb3}", name="m2")
        nc.vector.tensor_mul(m2[:], sig[:, :, 0:6], tg[:])
        nc.vector.tensor_add(c_cur[:, k0:k0 + nb3, :], m1[:], m2[:])
        tcn = work.tile([P, nb3, BSZ], F32, tag=f"tcn{nb3}", name="tcn")
        nc.scalar.activation(tcn[:], c_cur[:, k0:k0 + nb3, :], AF.Tanh)
        nc.vector.tensor_mul(h_cur[:, k0:k0 + nb3, :], sig[:, :, 12:18], tcn[:])

    def jit_z3x(dst, Hsrc):
        """Batched L3 input projection for HB steps: Wi3^T @ h2 + b3 from
        Hsrc [P, 4, HB, BSZ] (a completed hA/hB half-body) into dst
        [P, HB, 24].  Amortizes Wi3's 16 weight-chunk loads over HB steps
        (16/step -> 0.5/step) by making the moving operand HB*BSZ wide."""
        w = W[3]
        Hf = Hsrc[:, 0:4].rearrange("p c t b -> p c (t b)")
        for s in range(4):
            pt = jitp.tile([P, HB * BSZ], F32, tag="jit", name="pt3")
            for kc in range(4):
                idx = (s * 4 + kc) * P
                nc.tensor.matmul(
                    pt[:], w["wi"][:, idx:idx + P], Hf[:, kc, :],
                    start=(kc == 0), stop=(kc == 3))
            nc.vector.tensor_scalar_add(
                dst[:, 4, :, 6 * s:6 * s + 6],
                pt[:].rearrange("p (t b) -> p t b", b=BSZ),
                b3bc[:, 6 * s:6 * s + 1])

    def l3_mms(st):
        """The 4 Wh3 matmuls for fused-L3 (body-local) step st; h3 lives in
        block 4 of the h ring.  Slots 0,1 land in the half-B lo bank cols
        48:60, slots 2,3 in the hi bank cols 60:72 (complementary regions are
        zeroed once at TREP start and never written), so the L3 z-sum rides
        the half-B zs0/zsum adds with no extra instruction."""
        w = W[3]
        hp, _ = h_aps(st)
        for s in range(4):
            bank = zplo[1] if s < 2 else zphi[1]
            nc.tensor.matmul(
                bank[:, 48 + 6 * s:48 + 6 * s + 6], w["wh"][:, s * P:s * P + P],
                hp[:, 4, :], start=True, stop=True)

    def l3_step_full(q, z3x_ap):
        """Standalone L3 step (epilogue drain only): matmuls + gate chain for
        ring block 4.  L3 weights carry GSCALE like L1/L2."""
        l3_mms(q)
        hp, hc = h_aps(q)
        cp, cc = (cA, cB) if q % 2 == 0 else (cB, cA)
        zs0e = work.tile([P, 24], F32, tag="zs0e", name="zs0e")
        nc.vector.tensor_add(zs0e[:], zplo[1][:, 48:72], z3x_ap)
        zsum = work.tile([P, 24], F32, tag="zsum3", name="zsum3")
        nc.vector.tensor_add(zsum[:], zs0e[:], zphi[1][:, 48:72])
        sig = work.tile([P, 18], F32, tag="sig3", name="sig3")
        nc.scalar.activation(sig[:], zsum[:, 0:18], AF.Sigmoid, scale=DESCALE)
        tg = work.tile([P, BSZ], F32, tag="tg3", name="tg3")
        nc.scalar.activation(tg[:], zsum[:, 18:24], AF.Tanh, scale=DESCALE)
        m1 = work.tile([P, BSZ], F32, tag="m31", name="m31")
        nc.vector.tensor_mul(m1[:], sig[:, 6:12], cp[:, 4, :])
        m2 = work.tile([P, BSZ], F32, tag="m32", name="m32")
        nc.vector.tensor_mul(m2[:], sig[:, 0:6], tg[:])
        nc.vector.tensor_add(cc[:, 4, :], m1[:], m2[:])
        tcn = work.tile([P, BSZ], F32, tag="tc3", name="tc3")
        nc.scalar.activation(tcn[:], cc[:, 4, :], AF.Tanh)
        nc.vector.tensor_mul(hc[:, 4, :], sig[:, 12:18], tcn[:])

    def h_aps(st):
        cur = (hA if st < HB else hB)[:, :, st % HB, :]
        if st == 0:
            prev = hB[:, :, HB - 1, :]
        else:
            prev = (hA if st - 1 < HB else hB)[:, :, (st - 1) % HB, :]
        return prev, cur

    SKIP_GATES = os.environ.get("SKIP_GATES", "0") == "1"
    SKIP_MMS = os.environ.get("SKIP_MMS", "0") == "1"
    SIM_UNROLL = os.environ.get("SIM_UNROLL", "0") == "1"

    def loop(n, body):
        """tc.For_i hardware loop; full python unroll when SIM_UNROLL=1
        (TimelineSim can't take reg-mode branches)."""
        if SIM_UNROLL:
            for i in range(n):
                body(i)
        else:
            with tc.For_i(0, n, 1, hint_engines=(mybir.EngineType.PE, mybir.EngineType.DVE, mybir.EngineType.Activation)) as iv:
                body(iv)
    PH1 = int(os.environ.get("PH1", str(NBODY)))
    PH2 = int(os.environ.get("PH2", str(NBODY - 1)))

    def l3_ap(st):
        zbuf = zxR[0] if st < HB else zxR[1]
        return zbuf[:, 4, st % HB, :]

    def body_step(L, st, with_l3):
        hp, hc = h_aps(st)
        cp, cc = (cA, cB) if st % 2 == 0 else (cB, cA)
        zbuf = zxR[0] if st < HB else zxR[1]
        zx_ap = zbuf[:, :, st % HB, :]
        if not SKIP_MMS:
            step_mms(L, 0, st, hp)
        if not SKIP_GATES:
            gates_half(L, 0, st, hc, cp, cc, zx_ap)
        if not SKIP_MMS:
            step_mms(L, 1, st, hp)
            if with_l3:
                l3_mms(st)
        if not SKIP_GATES:
            gates_half(L, 1, st, hc, cp, cc, zx_ap,
                       z3x_ap=(zx_ap is not None and with_l3) or None)

    # ================= Phase 1: L1 =================
    if SKIP_GATES:
        nc.vector.memset(hA[:], 0.0)
        nc.vector.memset(hB[:], 0.0)
        nc.vector.memset(cB[:], 0.0)
    if SKIP_MMS:
        for t_ in zplo + zphi:
            nc.vector.memset(t_[:], 0.0)
    TREP = int(os.environ.get("TREP", "1"))

    def emit_body1(iv):
        t0 = iv * BODY
        for st in range(BODY):
            body_step(1, st, with_l3=False)
            if st == HB - 1:
                nc.sync.dma_start(
                    seq1T[:, :, bass.ds(t0 * BSZ, HB * BSZ)],
                    hA[:, 0:4].rearrange("p c t b -> p c (t b)"))
                jit_zx(1, zxR[0], t0 + BODY)
        nc.sync.dma_start(
            seq1T[:, :, bass.ds((t0 + HB) * BSZ, HB * BSZ)],
            hB[:, 0:4].rearrange("p c t b -> p c (t b)"))
        jit_zx(1, zxR[1], t0 + BODY + HB)

    def emit_body2(t0, with_l3):
        """One L2 body at offset t0; interleaved L3 runs one body behind,
        consuming z3R, which is re-jitted here as hA/hB halves complete."""
        for st in range(BODY):
            body_step(2, st, with_l3=with_l3)
            if st == HB - 1:
                jit_zx(2, zxR[0], t0 + BODY, Ssrc=S[0])
                nc.sync.dma_start(
                    S[0][:], seq1T[:, :, bass.ds((t0 + 2 * BODY) * BSZ, HB * BSZ)])
                jit_z3x(zxR[0], hA)
        jit_z3x(zxR[1], hB)
        jit_zx(2, zxR[1], t0 + BODY + HB, Ssrc=S[1])
        nc.sync.dma_start(
            S[1][:], seq1T[:, :, bass.ds((t0 + 2 * BODY + HB) * BSZ, HB * BSZ)])

    def trep_body(_trep_i):
        nc.vector.memset(hB[:, :, HB - 1, :], 0.0)
        nc.vector.memset(cA[:], 0.0)
        nc.vector.memset(zplo[1][:, 60:72], 0.0)
        nc.vector.memset(zphi[1][:, 48:60], 0.0)
        nc.vector.memset(zpad[:], 0.0)
        nc.sync.dma_start(
            seq1T[:, :, T * BSZ:Tpad * BSZ],
            zpad[:].rearrange("p (c t) -> p c t", c=4))
        jit_zx(1, zxR[0], 0)
        jit_zx(1, zxR[1], HB)

        loop(PH1, emit_body1)

        # ================= Phase 2: L2 + fused L3 =================
        nc.sync.dma_start(S[0][:], seq1T[:, :, 0:HB * BSZ])
        nc.sync.dma_start(S[1][:], seq1T[:, :, HB * BSZ:BODY * BSZ])
        jit_zx(2, zxR[0], 0, Ssrc=S[0])
        jit_zx(2, zxR[1], HB, Ssrc=S[1])
        nc.sync.dma_start(S[0][:], seq1T[:, :, BODY * BSZ:(BODY + HB) * BSZ])
        nc.sync.dma_start(S[1][:], seq1T[:, :, (BODY + HB) * BSZ:2 * BODY * BSZ])

        # L2 body 0 (prologue, no L3 yet -- L3 trails by one body)
        emit_body2(0, with_l3=False)
        loop(PH2, lambda iv: emit_body2(iv * BODY + BODY, with_l3=True))
        # L3 epilogue: drain the last body's steps
        for st in range(BODY):
            l3_step_full(st, l3_ap(st))

    loop(TREP, trep_body)

    # ================= Final linear =================
    out_ps = jitp.tile([1, BSZ], F32, tag="jit", name="out_ps")
    nc.tensor.matmul(out_ps[:], wl[:], hB[:, 4, HB - 1, :], start=True, stop=True)
    blt = work.tile([1, 1], F32, tag="blt", name="blt")
    nc.vector.memset(blt[:], bl_value)
    outsb = work.tile([1, BSZ], F32, tag="outsb", name="outsb")
    nc.scalar.activation(outsb[:], out_ps[:], AF.Identity, bias=blt[:])
    nc.sync.dma_start(outs["out"].rearrange("a b -> b a"), outsb[:])
    ctx.close()


def build_program(T=T_FULL, BODY=BODY_DEFAULT, bl_value=0.0, shapes=None):
    nc = bacc.Bacc("TRN2", target_bir_lowering=False, debug=False,
                   enable_asserts=False, num_devices=1)
    ins = {}
    for k, (shape, dtype) in shapes.items():
        ins[k] = nc.dram_tensor(k, list(shape), dtype, kind="ExternalInput").ap()
    out = nc.dram_tensor("out", [BSZ, 1], F32, kind="ExternalOutput").ap()
    with tile.TileContext(nc) as tc:
        build_lstm(tc, {"out": out}, ins, T, BODY, bl_value)
    nc.compile()
    return nc


def run(inputs, T=T_FULL, BODY=BODY_DEFAULT, trace=False):
    dev_in, bl_value = prep_inputs(inputs, T, BODY)
    shapes = {k: (v.shape, mybir.dt.from_np(v.dtype)) for k, v in dev_in.items()}
    nc = build_program(T=T, BODY=BODY, bl_value=bl_value, shapes=shapes)
    # The device occasionally wedges transiently (NRT_EXEC_UNIT_UNRECOVERABLE,
    # observed ~1/100 runs); a short retry recovers it.
    last_err = None
    for attempt in range(3):
        try:
            res = run_bass_kernel_spmd(nc, [dev_in], core_ids=[0], trace=trace)
            return res.results[0]["out"], res
        except Exception as e:  # noqa: BLE001 - retry any runtime failure
            last_err = e
            import time as _time
            _time.sleep(10 * (attempt + 1))
    raise last_err


def kernel(**inputs):
    inputs = {k: np.asarray(v) for k, v in inputs.items()}
    out, _ = run(inputs)
    return out.astype(np.float32)



# revision 33
# speedup vs baseline: 1.0257x; 1.0257x over previous
"""Self-contained Trainium2 Bass kernel for the 3-layer LSTM problem
(nn_CustomModel_16681652978184): T=4096, B=6, F=128, H1=512, H3=128.

Strategy (chosen over the sharding hint's per-step tensor-parallel option):
the recurrence is strictly serial (8192 dependent steps: L2's initial state
is L1's *final* state, so L1/L2 cannot pipeline), and cross-core exchange
floors on trn2 (~2us DMA fixed cost, ~5-10us collective floor) dwarf the
~4us per-step compute -- an "all-reduce h each step" design would spend
40ms+ in sync alone.  So the serial recurrence runs on ONE NeuronCore,
structured to make each step as fast as the PE weight-load bandwidth allows:

  - "Transposed land": activations live as [H-on-partitions, batch].
    Recurrent matmul z^T = Wh^T @ h^T with bf16 weight chunks stationary
    (Fast-Weight-Load) and tiny h^T [128, 6] moving operands.
  - Gate-column permutation: PSUM gate tiles hold (i | f | o | g) x batch
    per H-block, and gate math is emitted as single strided-AP instructions
    spanning all blocks of a half (sigmoid: one [128, nb/2, 18] ACT op) --
    the ACT fixed cost (~300ns/instr) makes many tiny ops ruinous.
  - Half-split software pipelining: the H-blocks are split in two halves;
    while the PE streams half B's matmuls, half A's gate chain runs on
    ACT/VEC, hiding the serial gate latency under the weight stream.
  - Input projections (x @ Wi + b) computed just-in-time inside the loop
    body (off the critical path) into SBUF ring buffers; only seq1 round
    trips through DRAM (25MB > SBUF).  L3 trails L2 by one body: its input
    projections (Wi3^T h2 + b3) are batched per half-body (jit_z3x, N=192
    moving operand -> 0.5 weight-chunk loads/step instead of 16), and only
    the 4 Wh3 chunks remain per-step; l3 epilogue drains the final body.
  - Wh1/Wh2 are fp8 e4m3 (trn2 float8e4: max finite 240) scaled by 2^12,
    halving the dominant LDWEIGHTS stream; Wi/b of L1/L2 carry the same
    scale (bf16/f32) and the descale rides the gate ACT scale= operand for
    free.  L3 + Wl stay bf16 (L3 quantization fails the 2e-2 gate).
    Measured rel err 1.1e-2 (gate 2e-2).
  - Dynamic For_i outer loops with unrolled bodies; parity-free ring
    buffers keep all inner addressing static.  TREP env knob wraps both
    phases in an outer repeat loop (state is re-zeroed each round) for
    overhead-cancelling slope timing; semantics are unchanged (TREP=1).
"""

import os
import numpy as np
import ml_dtypes

import concourse.bass as bass
import concourse.mybir as mybir
from concourse import bacc, tile
from concourse.bass_utils import run_bass_kernel_spmd

F32 = mybir.dt.float32
BF16 = mybir.dt.bfloat16
AF = mybir.ActivationFunctionType

P = 128
BSZ = 6

T_FULL = 4096
BODY_DEFAULT = 64

# Wh1/Wh2 are stored fp8 e4m3 (trn2 float8e4 = IEEE-ish: max finite 240,
# exp-15 encodes inf/nan) scaled by 2^12: |Wh|<=1/sqrt(512)*4096=181<240.
# Wi/b of those layers carry the same scale so zsum is uniformly scaled; the
# descale folds into the gate activations' scale= operand (zero extra instrs).
GSCALE = 2.0 ** 12
DESCALE = 2.0 ** -12

# slot -> reference gate column-block base multiplier (ref order i,f,g,o)
_SLOT_BASE = {0: 0, 1: 1, 2: 3, 3: 2}  # our slots: i, f, o, g


def gcol(H, kb, s):
    return _SLOT_BASE[s] * H + kb * P


def prep_layer(Wi, Wh, b, H, wh_fp8=False):
    bf = ml_dtypes.bfloat16
    nb = H // P
    KCi = Wi.shape[0] // P
    KCh = Wh.shape[0] // P
    scale = GSCALE
    wh_dt = ml_dtypes.float8_e4m3 if wh_fp8 else bf
    Wi = np.asarray(Wi) * scale
    Wh = np.asarray(Wh) * scale
    b = np.asarray(b) * scale
    WiP = np.zeros((P, nb * 4 * KCi * P), dtype=bf)
    WhP = np.zeros((P, nb * 4 * KCh * P), dtype=wh_dt)
    bP = np.zeros((P, nb * 4), dtype=np.float32)
    for kb in range(nb):
        for s in range(4):
            col = gcol(H, kb, s)
            bP[:, kb * 4 + s] = b[col:col + P]
            for kc in range(KCi):
                idx = ((kb * 4 + s) * KCi + kc) * P
                WiP[:, idx:idx + P] = Wi[kc * P:(kc + 1) * P, col:col + P].astype(bf)
            for kc in range(KCh):
                idx = ((kb * 4 + s) * KCh + kc) * P
                WhP[:, idx:idx + P] = Wh[kc * P:(kc + 1) * P, col:col + P].astype(wh_dt)
    return WiP, WhP, bP


def prep_inputs(inp, T, BODY):
    bf = ml_dtypes.bfloat16
    x = np.asarray(inp["x"])[:T]
    Tpad = T + 2 * BODY
    xT = np.zeros((P, Tpad * BSZ), dtype=bf)
    xT[:, : T * BSZ] = x.reshape(T * BSZ, P).T.astype(bf)

    Wi1P, Wh1P, b1P = prep_layer(inp["Wi1"], inp["Wh1"], inp["b1"], 512, wh_fp8=True)
    Wi2P, Wh2P, b2P = prep_layer(inp["Wi2"], inp["Wh2"], inp["b2"], 512, wh_fp8=True)
    Wi3P, Wh3P, b3P = prep_layer(inp["Wi3"], inp["Wh3"], inp["b3"], 128)
    # broadcast b3 over batch for the fused-L3 gate add: [128, 4slots*6]
    b3bc = np.repeat(b3P[:, 0:4], BSZ, axis=1).astype(np.float32)
    WlP = np.asarray(inp["Wl"]).astype(bf)
    return {
        "xT": xT,
        "Wi1P": Wi1P, "Wh1P": Wh1P, "b1P": b1P,
        "Wi2P": Wi2P, "Wh2P": Wh2P, "b2P": b2P,
        "Wi3P": Wi3P, "Wh3P": Wh3P, "b3bc": b3bc,
        "WlP": WlP,
    }, float(np.asarray(inp["bl"])[0])


def build_lstm(tc, outs, ins, T, BODY, bl_value):
    nc = tc.nc
    assert T % BODY == 0 and BODY % 2 == 0
    HB = BODY // 2
    NBODY = T // BODY
    Tpad = T + 2 * BODY

    from contextlib import ExitStack
    ctx = ExitStack()
    const = ctx.enter_context(tc.tile_pool(name="const", bufs=1))
    state = ctx.enter_context(tc.tile_pool(name="state", bufs=1))
    ppool = ctx.enter_context(tc.tile_pool(name="ppool", bufs=1, space=bass.MemorySpace.PSUM))
    jitp = ctx.enter_context(tc.tile_pool(name="jitp", bufs=2, space=bass.MemorySpace.PSUM))
    dram = ctx.enter_context(tc.tile_pool(name="dram", bufs=1, space=bass.MemorySpace.DRAM))
    work = ctx.enter_context(tc.tile_pool(name="work", bufs=4))

    def load_const(key, shape, dtype):
        t = const.tile(shape, dtype, tag=key, name=key)
        nc.sync.dma_start(t[:], ins[key])
        return t

    xT = load_const("xT", [P, Tpad * BSZ], BF16)
    F8 = mybir.dt.float8e4
    W = {}
    for L, KCi, KCh, nb in ((1, 1, 4, 4), (2, 4, 4, 4), (3, 4, 1, 1)):
        W[L] = dict(
            wi=load_const(f"Wi{L}P", [P, nb * 4 * KCi * P], BF16),
            wh=load_const(f"Wh{L}P", [P, nb * 4 * KCh * P], F8 if L != 3 else BF16),
            KCi=KCi, KCh=KCh, nb=nb,
        )
    W[1]["b"] = load_const("b1P", [P, 16], F32)
    W[2]["b"] = load_const("b2P", [P, 16], F32)
    b3bc = load_const("b3bc", [P, 24], F32)
    wl = load_const("WlP", [P, 1], BF16)

    # block 4 of h/c = the fused L3 recurrence (one body behind L2): its gate
    # math rides the half-B strided instructions for free.
    hA = state.tile([P, 5, HB, BSZ], BF16, tag="hA")
    hB = state.tile([P, 5, HB, BSZ], BF16, tag="hB")
    cA = state.tile([P, 5, BSZ], F32, tag="cA")
    cB = state.tile([P, 5, BSZ], F32, tag="cB")
    zxR = [state.tile([P, 4, HB, 24], F32, tag=f"zxR{i}", name=f"zxR{i}") for i in range(2)]
    z3R = [state.tile([P, HB, 24], F32, tag=f"z3R{i}", name=f"z3R{i}") for i in range(2)]
    S = [state.tile([P, 4, HB * BSZ], BF16, tag=f"S{i}", name=f"S{i}") for i in range(2)]
    zpad = state.tile([P, 4 * 2 * BODY * BSZ], BF16, tag="zpad")

    # PSUM: 2 halves x (lo=kc01 | hi=kc23) + L3 + jit pool (2) = 7 banks.
    # lo/hi split keeps accumulation groups consecutive (interleaved groups
    # corrupt PSUM) while letting the first 16 MMs of a step depend only on
    # the previous step's first-half h.  (Merging lo+hi into single 4-chunk
    # groups was tried: +10ms -- the early-start overlap is load-bearing.)
    zplo = [ppool.tile([P, 48], F32, tag=f"zplo{h}", name=f"zplo{h}") for h in (0, 1)]
    zphi = [ppool.tile([P, 48], F32, tag=f"zphi{h}", name=f"zphi{h}") for h in (0, 1)]
    z3p = ppool.tile([P, 24], F32, tag="z3p", name="z3p")

    seq1T = dram.tile([P, 4, Tpad * BSZ], BF16, tag="seq1T")

    # =====================================================================
    def jit_zx(L, dst, base, Ssrc=None):
        """zx (= Wi^T @ input + b) for HB steps starting at absolute step
        `base` (int or ScalarValue) into dst [P, nb, HB, 24] (bf16)."""
        w = W[L]
        for kb in range(w["nb"]):
            for s in range(4):
                pt = jitp.tile([P, HB * BSZ], F32, tag="jit", name="jit")
                for kc in range(w["KCi"]):
                    if L == 1:
                        rhs = xT[:, bass.ds(base * BSZ, HB * BSZ)]
                    else:
                        rhs = Ssrc[:, kc, :]
                    idx = ((kb * 4 + s) * w["KCi"] + kc) * P
                    nc.tensor.matmul(
                        pt[:], w["wi"][:, idx:idx + P], rhs,
                        start=(kc == 0), stop=(kc == w["KCi"] - 1))
                nc.vector.tensor_scalar_add(
                    dst[:, kb, :, 6 * s:6 * s + 6],
                    pt[:].rearrange("p (t b) -> p t b", b=BSZ),
                    w["b"][:, kb * 4 + s:kb * 4 + s + 1])

    def step_mms(L, half, st, h_prev):
        """PE stream for one half of step st, split into lo (kc 0-1) and hi
        (kc 2-3) accumulators so the lo block only needs h-blocks 0-1 of the
        previous step (whose gate chain finished earliest)."""
        w = W[L]
        KCh = w["KCh"]
        assert KCh == 4
        groups = [(zplo[half], (0, 1)), (zphi[half], (2, 3))]
        for zp, kcs in groups:
            for kb in (half * 2, half * 2 + 1):
                for s in range(4):
                    o = 24 * (kb - half * 2) + 6 * s
                    for j, kc in enumerate(kcs):
                        idx = ((kb * 4 + s) * KCh + kc) * P
                        nc.tensor.matmul(
                            zp[:, o:o + 6],
                            w["wh"][:, idx:idx + P],
                            h_prev[:, kc, :],
                            start=(j == 0), stop=(j == len(kcs) - 1))

    def gates_half(L, half, st, h_cur, c_prev, c_cur, zx_ap, z3x_ap=None):
        """Gate math for blocks [2*half, 2*half+2) of step st, merged into
        strided single instructions.  When z3x_ap is given (fused L3 on
        half 1), a third block rides every instruction: slot 2 of the zsum
        tile is filled from z3p + z3x and blocks [2:5) of h/c are updated."""
        k0 = half * 2
        nb3 = 3 if z3x_ap is not None else 2
        lo3 = zplo[half][:].rearrange("p (k g) -> p k g", g=24)
        hi3 = zphi[half][:].rearrange("p (k g) -> p k g", g=24)
        zsum = work.tile([P, nb3, 24], F32, tag=f"zsum{nb3}", name="zsum")
        zs0 = work.tile([P, 2, 24], F32, tag="zs0", name="zs0")
        nc.vector.tensor_add(zs0[:], lo3, zx_ap[:, k0:k0 + 2, :])
        nc.vector.tensor_add(zsum[:, 0:2, :], zs0[:], hi3)
        if z3x_ap is not None:
            nc.vector.tensor_add(zsum[:, 2, :], z3p[:], z3x_ap)
        sig = work.tile([P, nb3, 18], F32, tag=f"sig{nb3}", name="sig")
        nc.scalar.activation(sig[:], zsum[:, :, 0:18], AF.Sigmoid, scale=DESCALE)
        tg = work.tile([P, nb3, BSZ], F32, tag=f"tg{nb3}", name="tg")
        nc.scalar.activation(tg[:], zsum[:, :, 18:24], AF.Tanh, scale=DESCALE)
        m1 = work.tile([P, nb3, BSZ], F32, tag=f"m1{nb3}", name="m1")
        nc.vector.tensor_mul(m1[:], sig[:, :, 6:12], c_prev[:, k0:k0 + nb3, :])
        m2 = work.tile([P, nb3, BSZ], F32, tag=f"m2{nb3}", name="m2")
        nc.vector.tensor_mul(m2[:], sig[:, :, 0:6], tg[:])
        nc.vector.tensor_add(c_cur[:, k0:k0 + nb3, :], m1[:], m2[:])
        tcn = work.tile([P, nb3, BSZ], F32, tag=f"tcn{nb3}", name="tcn")
        nc.scalar.activation(tcn[:], c_cur[:, k0:k0 + nb3, :], AF.Tanh)
        nc.vector.tensor_mul(h_cur[:, k0:k0 + nb3, :], sig[:, :, 12:18], tcn[:])

    def jit_z3x(dst, Hsrc):
        """Batched L3 input projection for HB steps: Wi3^T @ h2 + b3 from
        Hsrc [P, 4, HB, BSZ] (a completed hA/hB half-body) into dst
        [P, HB, 24].  Amortizes Wi3's 16 weight-chunk loads over HB steps
        (16/step -> 0.5/step) by making the moving operand HB*BSZ wide."""
        w = W[3]
        Hf = Hsrc[:, 0:4].rearrange("p c t b -> p c (t b)")
        for s in range(4):
            pt = jitp.tile([P, HB * BSZ], F32, tag="jit", name="pt3")
            for kc in range(4):
                idx = (s * 4 + kc) * P
                nc.tensor.matmul(
                    pt[:], w["wi"][:, idx:idx + P], Hf[:, kc, :],
                    start=(kc == 0), stop=(kc == 3))
            nc.vector.tensor_scalar_add(
                dst[:, :, 6 * s:6 * s + 6],
                pt[:].rearrange("p (t b) -> p t b", b=BSZ),
                b3bc[:, 6 * s:6 * s + 1])

    def l3_mms(st):
        """The 4 Wh3 matmuls for fused-L3 (body-local) step st; h3 lives in
        block 4 of the h ring."""
        w = W[3]
        hp, _ = h_aps(st)
        for s in range(4):
            nc.tensor.matmul(
                z3p[:, 6 * s:6 * s + 6], w["wh"][:, s * P:s * P + P],
                hp[:, 4, :], start=True, stop=True)

    def l3_step_full(q, z3x_ap):
        """Standalone L3 step (epilogue drain only): matmuls + gate chain for
        ring block 4.  L3 weights carry GSCALE like L1/L2."""
        l3_mms(q)
        hp, hc = h_aps(q)
        cp, cc = (cA, cB) if q % 2 == 0 else (cB, cA)
        zsum = work.tile([P, 24], F32, tag="zsum3", name="zsum3")
        nc.vector.tensor_add(zsum[:], z3p[:], z3x_ap)
        sig = work.tile([P, 18], F32, tag="sig3", name="sig3")
        nc.scalar.activation(sig[:], zsum[:, 0:18], AF.Sigmoid, scale=DESCALE)
        tg = work.tile([P, BSZ], F32, tag="tg3", name="tg3")
        nc.scalar.activation(tg[:], zsum[:, 18:24], AF.Tanh, scale=DESCALE)
        m1 = work.tile([P, BSZ], F32, tag="m31", name="m31")
        nc.vector.tensor_mul(m1[:], sig[:, 6:12], cp[:, 4, :])
        m2 = work.tile([P, BSZ], F32, tag="m32", name="m32")
        nc.vector.tensor_mul(m2[:], sig[:, 0:6], tg[:])
        nc.vector.tensor_add(cc[:, 4, :], m1[:], m2[:])
        tcn = work.tile([P, BSZ], F32, tag="tc3", name="tc3")
        nc.scalar.activation(tcn[:], cc[:, 4, :], AF.Tanh)
        nc.vector.tensor_mul(hc[:, 4, :], sig[:, 12:18], tcn[:])

    def h_aps(st):
        cur = (hA if st < HB else hB)[:, :, st % HB, :]
        if st == 0:
            prev = hB[:, :, HB - 1, :]
        else:
            prev = (hA if st - 1 < HB else hB)[:, :, (st - 1) % HB, :]
        return prev, cur

    SKIP_GATES = os.environ.get("SKIP_GATES", "0") == "1"
    SKIP_MMS = os.environ.get("SKIP_MMS", "0") == "1"
    SIM_UNROLL = os.environ.get("SIM_UNROLL", "0") == "1"

    def loop(n, body):
        """tc.For_i hardware loop; full python unroll when SIM_UNROLL=1
        (TimelineSim can't take reg-mode branches)."""
        if SIM_UNROLL:
            for i in range(n):
                body(i)
        else:
            with tc.For_i(0, n, 1, hint_engines=(mybir.EngineType.PE, mybir.EngineType.DVE, mybir.EngineType.Activation)) as iv:
                body(iv)
    PH1 = int(os.environ.get("PH1", str(NBODY)))
    PH2 = int(os.environ.get("PH2", str(NBODY - 1)))

    def l3_ap(st):
        return z3R[0][:, st, :] if st < HB else z3R[1][:, st - HB, :]

    def body_step(L, st, with_l3):
        hp, hc = h_aps(st)
        cp, cc = (cA, cB) if st % 2 == 0 else (cB, cA)
        zbuf = zxR[0] if st < HB else zxR[1]
        zx_ap = zbuf[:, :, st % HB, :]
        if not SKIP_MMS:
            step_mms(L, 0, st, hp)
        if not SKIP_GATES:
            gates_half(L, 0, st, hc, cp, cc, zx_ap)
        if not SKIP_MMS:
            step_mms(L, 1, st, hp)
            if with_l3:
                l3_mms(st)
        if not SKIP_GATES:
            gates_half(L, 1, st, hc, cp, cc, zx_ap,
                       z3x_ap=l3_ap(st) if with_l3 else None)

    # ================= Phase 1: L1 =================
    if SKIP_GATES:
        nc.vector.memset(hA[:], 0.0)
        nc.vector.memset(hB[:], 0.0)
        nc.vector.memset(cB[:], 0.0)
    if SKIP_MMS:
        for t_ in zplo + zphi + [z3p]:
            nc.vector.memset(t_[:], 0.0)
    TREP = int(os.environ.get("TREP", "1"))

    def emit_body1(iv):
        t0 = iv * BODY
        for st in range(BODY):
            body_step(1, st, with_l3=False)
            if st == HB - 1:
                nc.sync.dma_start(
                    seq1T[:, :, bass.ds(t0 * BSZ, HB * BSZ)],
                    hA[:, 0:4].rearrange("p c t b -> p c (t b)"))
                jit_zx(1, zxR[0], t0 + BODY)
        nc.sync.dma_start(
            seq1T[:, :, bass.ds((t0 + HB) * BSZ, HB * BSZ)],
            hB[:, 0:4].rearrange("p c t b -> p c (t b)"))
        jit_zx(1, zxR[1], t0 + BODY + HB)

    def emit_body2(t0, with_l3):
        """One L2 body at offset t0; interleaved L3 runs one body behind,
        consuming z3R, which is re-jitted here as hA/hB halves complete."""
        for st in range(BODY):
            body_step(2, st, with_l3=with_l3)
            if st == HB - 1:
                jit_zx(2, zxR[0], t0 + BODY, Ssrc=S[0])
                nc.sync.dma_start(
                    S[0][:], seq1T[:, :, bass.ds((t0 + 2 * BODY) * BSZ, HB * BSZ)])
                jit_z3x(z3R[0], hA)
        jit_z3x(z3R[1], hB)
        jit_zx(2, zxR[1], t0 + BODY + HB, Ssrc=S[1])
        nc.sync.dma_start(
            S[1][:], seq1T[:, :, bass.ds((t0 + 2 * BODY + HB) * BSZ, HB * BSZ)])

    def trep_body(_trep_i):
        nc.vector.memset(hB[:, :, HB - 1, :], 0.0)
        nc.vector.memset(cA[:], 0.0)
        nc.vector.memset(zpad[:], 0.0)
        nc.sync.dma_start(
            seq1T[:, :, T * BSZ:Tpad * BSZ],
            zpad[:].rearrange("p (c t) -> p c t", c=4))
        jit_zx(1, zxR[0], 0)
        jit_zx(1, zxR[1], HB)

        loop(PH1, emit_body1)

        # ================= Phase 2: L2 + fused L3 =================
        nc.sync.dma_start(S[0][:], seq1T[:, :, 0:HB * BSZ])
        nc.sync.dma_start(S[1][:], seq1T[:, :, HB * BSZ:BODY * BSZ])
        jit_zx(2, zxR[0], 0, Ssrc=S[0])
        jit_zx(2, zxR[1], HB, Ssrc=S[1])
        nc.sync.dma_start(S[0][:], seq1T[:, :, BODY * BSZ:(BODY + HB) * BSZ])
        nc.sync.dma_start(S[1][:], seq1T[:, :, (BODY + HB) * BSZ:2 * BODY * BSZ])

        # L2 body 0 (prologue, no L3 yet -- L3 trails by one body)
        emit_body2(0, with_l3=False)
        loop(PH2, lambda iv: emit_body2(iv * BODY + BODY, with_l3=True))
        # L3 epilogue: drain the last body's steps
        for st in range(BODY):
            l3_step_full(st, l3_ap(st))

    loop(TREP, trep_body)

    # ================= Final linear =================
    out_ps = jitp.tile([1, BSZ], F32, tag="jit", name="out_ps")
    nc.tensor.matmul(out_ps[:], wl[:], hB[:, 4, HB - 1, :], start=True, stop=True)
    blt = work.tile([1, 1], F32, tag="blt", name="blt")
    nc.vector.memset(blt[:], bl_value)
    outsb = work.tile([1, BSZ], F32, tag="outsb", name="outsb")
    nc.scalar.activation(outsb[:], out_ps[:], AF.Identity, bias=blt[:])
    nc.sync.dma_start(outs["out"].rearrange("a b -> b a"), outsb[:])
    ctx.close()


def build_program(T=T_FULL, BODY=BODY_DEFAULT, bl_value=0.0, shapes=None):
    nc = bacc.Bacc("TRN2", target_bir_lowering=False, debug=False,
                   enable_asserts=False, num_devices=1)
    ins = {}
    for k, (shape, dtype) in shapes.items():
        ins[k] = nc.dram_tensor(k, list(shape), dtype, kind="ExternalInput").ap()
    out = nc.dram_tensor("out", [BSZ, 1], F32, kind="ExternalOutput").ap()
    with tile.TileContext(nc) as tc:
        build_lstm(tc, {"out": out}, ins, T, BODY, bl_value)
    nc.compile()
    return nc


def run(inputs, T=T_FULL, BODY=BODY_DEFAULT, trace=False):
    dev_in, bl_value = prep_inputs(inputs, T, BODY)
    shapes = {k: (v.shape, mybir.dt.from_np(v.dtype)) for k, v in dev_in.items()}
    nc = build_program(T=T, BODY=BODY, bl_value=bl_value, shapes=shapes)
    # The device occasionally wedges transiently (NRT_EXEC_UNIT_UNRECOVERABLE,
    # observed ~1/100 runs); a short retry recovers it.
    last_err = None
    for attempt in range(3):
        try:
            res = run_bass_kernel_spmd(nc, [dev_in], core_ids=[0], trace=trace)
            return res.results[0]["out"], res
        except Exception as e:  # noqa: BLE001 - retry any runtime failure
            last_err = e
            import time as _time
            _time.sleep(10 * (attempt + 1))
    raise last_err


def kernel(**inputs):
    inputs = {k: np.asarray(v) for k, v in inputs.items()}
    out, _ = run(inputs)
    return out.astype(np.float32)

